# revision 15
# baseline (speedup 1.0000x reference)
"""CBOW (nn_CBOW_88991722373900) Trainium2 kernel, v2.

Full-input contract: kernel(context_words[10,128000] f32, W_in[300,128000] f32,
W_out[128000,300] f32) -> softmax probabilities [128000] f32.

Strategy (8-way tensor/model parallel over the vocab dim V):
  - shard V into 8 chunks of 16000; each core holds its slice of both weight
    matrices cast to fp8e4 on host (memory-bound problem: quarter of the f32
    HBM traffic). Weights are pre-scaled by powers of two (S1, S2) so the
    fp8 values sit in the normal range; descales fold into existing scalar
    ops on the f32 PSUM results.
  - ctx pre-reduced over the C=10 window on DVE -> s[128,125] bf16, so
    GEMM1 is s-chunk (stationary, [128,1] bf16) x W_inT tile (moving fp8,
    [128,300]) accumulated over 125 v-chunks into PSUM [1,300].
  - trigger path: PSUM->SBUF bf16 copy -> gpsimd DMA -> AllGather(600B)
    (gpsimd carries only collective-path + output traffic, so the trigger
    fires as soon as GEMM1 retires).
  - post-AG rank-sum directly on PE in both layouts GEMM2 needs:
      h_nt  [128,3]  = hall-chunk (stationary) x ones8 (moving)   (n on parts)
      h_rep [128,300] = ones[8,128] (stationary) x hall (moving)  (broadcast)
    exact f32 1/(C*S1) on the PSUM->SBUF copies.
  - GEMM2 split across PE and DVE:
      PE  (v-blocks 0..PEB):  w2p fp8 col-blocks stationary (FWL), h_nt bf16
          moving -> PSUM [128, PEB]
      DVE (v-blocks PEB..125): one fused tensor_tensor_reduce per block:
          (w2r fp8 * h_rep bf16) summed over n -> lg_dve [128, DVB] f32
    v mapped so partition p holds contiguous v = 125*p + b
  - softmax: exp on ScalarE with scale=1/S2 (no max subtraction: |logit| < ~1
    at these weight scales), local sum via ones-matmul, AllGather(4B) for the
    global denominator, scale, DMA out.
"""

import numpy as np
import ml_dtypes

import concourse.bass as bass
import concourse.mybir as mybir
from concourse import tile
from concourse.bass_utils import run_bass_kernel_spmd
from concourse.vector_clock import ScopedClock, VectorClock

V = 128000
N = 300
C = 10
W = 8              # cores
VL = V // W        # 16000 vocab per core
NJ = VL // 128     # 125 v-chunks for GEMM1
NB = VL // 128     # 125 v-blocks for GEMM2
NCH = [(0, 128), (128, 128), (256, 44)]  # n-chunks
PEB = 80           # v-blocks on the PE lane of GEMM2
DVB = 35           # v-blocks on the DVE lane
GPB = NB - PEB - DVB  # v-blocks on the GpSimd lane
S1 = 4096.0        # host pre-scale on W_in  (values ~2.8e-3 -> ~11.4)
S2 = 128.0         # host pre-scale on W_out (values ~0.058  -> ~7.4)

BF16 = mybir.dt.bfloat16
F32 = mybir.dt.float32
FP8 = mybir.dt.float8e4
NP_BF16 = ml_dtypes.bfloat16
NP_FP8 = ml_dtypes.float8_e4m3fn

# w1 DMA groups: first ones small for an early GEMM1 start
W1_GROUPS = [4, 6] + [10] * 11 + [5]  # sums to 125
# DVE lane: one grouped-3D chunk (A) + per-block 2D chunks (B) to compare
# styles in the trace; GP lane: grouped-3D on gpsimd.
DVE_G3D = 13       # DVE blocks done grouped-3D
W2R_GROUPS = [13, 11, 11, 5, 5]       # DMA groups; sums to DVB + GPB


def _patched_drain_and_barrier(self, tick_clock, wait_clock):
    """Tail-drain waits split into 1-wait NOPs: this walrus build's CTRL
    instructions only encode a single sync wait."""
    vc = tick_clock.global_clock
    procs = [(p, vc[p]) for p in range(len(vc)) if vc[p] > 0]
    for i, (p, t) in enumerate(procs):
        pvc = VectorClock([0] * len(vc))
        pvc.require_at_least(p, t)
        nop_inst = self.nc.sync.nop(nofuse=True, hint=f"tail_wait_{i}")
        wait_clock.add_sem_waits(nop_inst.ins, ScopedClock({None: pvc}))
    self.nc.sync.drain()
    self.nc.all_engine_barrier(sem_only=True)
    assert self.sems is not None
    popped = self.nc._tile_sem_poison_stack.pop()
    assert popped is self._sem_poison
    self.nc.clear_and_free_semaphores(list(self.sems.allocated().values()))
    self.nc.all_engine_barrier(sem_only=True)


tile.TileContext._drain_and_barrier = _patched_drain_and_barrier


def _split_multi_waits(nc):
    """This walrus build encodes at most ONE sync wait per instruction. Hoist
    excess waits onto same-engine NoOps inserted immediately before."""
    import bass_rust

    ctr = [0]

    def make_nop(engine, wait):
        ctr[0] += 1
        nop = mybir.InstNoOp(name=f"I-wsplit{ctr[0]}", engine=engine)
        nop.bass_nofuse = True
        nop.sync_info = bass_rust.SyncInfo(on_wait=[wait], on_update=[])
        nc.register_instruction(nop, overwrite=True)
        return nop

    for bb in nc.main_func.blocks:
        out = []
        for ins in bb.instructions:
            si = ins.sync_info
            if si is not None and si.on_wait and len(si.on_wait) > 1:
                waits = list(si.on_wait)
                for w in waits[:-1]:
                    out.append(make_nop(ins.engine, w))
                ins.sync_info = bass_rust.SyncInfo(
                    on_wait=[waits[-1]], on_update=list(si.on_update)
                )
            out.append(ins)
        bb.instructions = out


def build_kernel():
    nc = bass.Bass()

    ctxp = nc.dram_tensor("ctxp", [128, NJ * C], BF16, kind="ExternalInput")
    # w1t packed partition-major on host: w1t[p, j*N + n] = W_in[n, v0+128j+p]*S1
    w1t = nc.dram_tensor("w1t", [128, NJ * N], FP8, kind="ExternalInput")
    # w2p: PE half, w2p[n, 128b+p] = W_out[v0+125p+b, n]*S2, b in [0, PEB)
    w2p = nc.dram_tensor("w2p", [N, PEB * 128], FP8, kind="ExternalInput")
    # w2r: DVE+GP lanes, w2r[p, bb*N+n] = W_out[v0+125p+PEB+bb, n]*S2
    w2r = nc.dram_tensor("w2r", [128, (DVB + GPB) * N], FP8, kind="ExternalInput")
    y_out = nc.dram_tensor("y", [128, NB], F32, kind="ExternalOutput")

    with tile.TileContext(nc) as tc:
        with (
            tc.tile_pool(name="const", bufs=1) as cpool,
            tc.tile_pool(name="scr", bufs=2) as spool,
            tc.tile_pool(name="psum", bufs=1, space="PSUM") as ppool,
            tc.tile_pool(name="dram", bufs=1, space="DRAM") as dpool,
        ):
            # ---- constants ----
            ones8 = cpool.tile([W, 1], BF16, tag="ones8")
            nc.vector.memset(ones8[:, :], 1.0)
            ones8r = cpool.tile([W, 128], BF16, tag="ones8r")
            nc.vector.memset(ones8r[:, :], 1.0)
            ones128 = cpool.tile([128, 1], F32, tag="ones128")
            nc.vector.memset(ones128[:, :], 1.0)

            # ---- input DMA streams ----
            # W_in (and ctx) strictly ahead of W_out on both rings so every
            # core's GEMM1 inputs win HBM arbitration fleet-wide; W_out
            # streams during the AllGather window.
            ctx_sb = cpool.tile([128, NJ * C], BF16, tag="ctx")
            JH = 63  # ctx/s split point
            nc.sync.dma_start(ctx_sb[:, 0:JH * C], ctxp[:, 0:JH * C])
            nc.scalar.dma_start(ctx_sb[:, JH * C:], ctxp[:, JH * C:])

            w1_sb = []
            j0 = 0
            for g, nj in enumerate(W1_GROUPS):
                t = cpool.tile([128, nj * N], FP8, tag=f"w1_{g}")
                ring = nc.scalar if g % 2 == 0 else nc.sync
                ring.dma_start(t[:, :], w1t[:, j0 * N:(j0 + nj) * N])
                w1_sb.append((t, j0, nj))
                j0 += nj

            w2p_sb = []
            for i3, (off, kk) in enumerate(NCH):
                t = cpool.tile([kk, PEB * 128], FP8, tag=f"w2p_{i3}")
                ring = nc.sync if i3 == 1 else nc.scalar
                ring.dma_start(t[:, :], w2p[off:off + kk, :])
                w2p_sb.append(t)

            w2r_sb = []
            bb0 = 0
            for g, nb in enumerate(W2R_GROUPS):
                t = cpool.tile([128, nb * N], FP8, tag=f"w2r_{g}")
                ring = nc.sync if g % 2 == 0 else nc.scalar
                ring.dma_start(t[:, :], w2r[:, bb0 * N:(bb0 + nb) * N])
                w2r_sb.append((t, bb0, nb))
                bb0 += nb

            # ---- ctx pre-reduce over C -> s[128, 125] bf16 ----
            s_sb = cpool.tile([128, NJ], BF16, tag="s")
            with nc.allow_low_precision(reason="C=10 window sum in bf16"):
                nc.vector.tensor_reduce(
                    s_sb[:, 0:JH],
                    ctx_sb[:, 0:JH * C].rearrange("p (j c) -> p j c", j=JH),
                    mybir.AxisListType.X,
                    mybir.AluOpType.add,
                )
                nc.vector.tensor_reduce(
                    s_sb[:, JH:],
                    ctx_sb[:, JH * C:].rearrange("p (j c) -> p j c", j=NJ - JH),
                    mybir.AxisListType.X,
                    mybir.AluOpType.add,
                )

            # ---- GEMM1: psum_hl[0, n] += s_j^T x w1 tile ----
            psum_hl = ppool.tile([1, N], F32, tag="phl")
            for t, j0g, nj in w1_sb:
                for jj in range(nj):
                    j = j0g + jj
                    nc.tensor.matmul(
                        psum_hl[:, :],
                        s_sb[:, j:j + 1],
                        t[:, jj * N:(jj + 1) * N],
                        start=(j == 0),
                        stop=(j == NJ - 1),
                    )

            # local partial hidden (scaled by S1*C) -> AllGather 600B.
            # cc_in DMA rides the idle vector ring; gpsimd's first duty is
            # the trigger itself so it fires the moment the DMA lands.
            h_loc = cpool.tile([1, N], BF16, tag="hloc")
            nc.vector.tensor_copy(h_loc[:, :], psum_hl[:, :])

            cc_in = dpool.tile([1, N], BF16, tag="cc_in")
            cc_out = dpool.tile([W, N], BF16, tag="cc_out")
            nc.gpsimd.dma_start(cc_in[:, :], h_loc[:, :])
            nc.gpsimd.collective_compute(
                "AllGather",
                mybir.AluOpType.bypass,
                replica_groups=[list(range(W))],
                ins=[cc_in.opt()],
                outs=[cc_out.opt()],
            )
            hall = cpool.tile([W, N], BF16, tag="hall")
            nc.gpsimd.dma_start(hall[:, :], cc_out[:, :])

            # ---- rank-sum on PE, directly in both layouts GEMM2 needs ----
            # h_nt: n on partitions ([128, 3] col per n-chunk)
            psum_t = ppool.tile([128, 3], F32, tag="pt")
            for i3, (off, kk) in enumerate(NCH):
                nc.tensor.matmul(
                    psum_t[0:kk, i3:i3 + 1],
                    hall[:, off:off + kk],
                    ones8[:, :],
                    start=True,
                    stop=True,
                )
            # h_rep: partition-broadcast of the rank sum
            psum_r = ppool.tile([128, N], F32, tag="pr")
            nc.tensor.matmul(psum_r[:, :], ones8r[:, :], hall[:, :])

            h_nt = cpool.tile([128, 3], BF16, tag="hnt")
            nc.vector.tensor_scalar_mul(h_nt[:, :], psum_t[:, :], 1.0 / (C * S1))
            h_rep = cpool.tile([128, N], BF16, tag="hrep")
            nc.scalar.activation(
                h_rep[:, :],
                psum_r[:, :],
                mybir.ActivationFunctionType.Copy,
                scale=1.0 / (C * S1),
            )

            # ---- GEMM2 PE half: logits[p, b]*S2 for b in [0, PEB) ----
            psum_l = ppool.tile([128, PEB], F32, tag="pl")
            for b in range(PEB):
                for i3, (off, kk) in enumerate(NCH):
                    nc.tensor.matmul(
                        psum_l[:, b:b + 1],
                        w2p_sb[i3][:, b * 128:(b + 1) * 128],
                        h_nt[0:kk, i3:i3 + 1],
                        start=(i3 == 0),
                        stop=(i3 == 2),
                    )

            # ---- GEMM2 DVE + GpSimd lanes over lg[128, DVB+GPB] ----
            # DVE lane: first DVE_G3D blocks grouped-3D, rest per-block 2D
            # (style A/B comparison in the trace); GP lane: grouped-3D.
            lg = cpool.tile([128, DVB + GPB], F32, tag="lg")
            h_b = h_rep[:, :].rearrange("p (x n) -> p x n", x=1)
            for t, bb0, nb in w2r_sb:
                if bb0 >= DVB:
                    # GP lane: mul on gpsimd, reduce via ScalarE accum_out
                    for bb in range(nb):
                        scr = spool.tile([128, N], BF16, tag="gp_scr")
                        nc.gpsimd.tensor_mul(
                            scr[:, :], t[:, bb * N:(bb + 1) * N], h_rep[:, :]
                        )
                        dump = spool.tile([128, N], BF16, tag="gp_dump")
                        nc.scalar.activation(
                            dump[:, :],
                            scr[:, :],
                            mybir.ActivationFunctionType.Copy,
                            accum_out=lg[:, bb0 + bb:bb0 + bb + 1],
                        )
                elif bb0 < DVE_G3D:
                    scr = spool.tile([128, nb * N], BF16, tag="g3d_scr")
                    nc.vector.tensor_mul(
                        scr[:, :].rearrange("p (b n) -> p b n", b=nb),
                        t[:, 0:nb * N].rearrange("p (b n) -> p b n", b=nb),
                        h_b.broadcast_to([128, nb, N]),
                    )
                    nc.vector.tensor_reduce(
                        lg[:, bb0:bb0 + nb],
                        scr[:, :].rearrange("p (b n) -> p b n", b=nb),
                        mybir.AxisListType.X,
                        mybir.AluOpType.add,
                    )
                else:
                    for bb in range(nb):
                        scr = spool.tile([128, N], BF16, tag="b2d_scr")
                        nc.vector.tensor_mul(
                            scr[:, :], t[:, bb * N:(bb + 1) * N], h_rep[:, :]
                        )
                        nc.vector.tensor_reduce(
                            lg[:, bb0 + bb:bb0 + bb + 1],
                            scr[:, :],
                            mybir.AxisListType.X,
                            mybir.AluOpType.add,
                        )

            # ---- softmax ----
            e_sb = cpool.tile([128, NB], F32, tag="esb")
            esum2 = cpool.tile([128, 2], F32, tag="esum2")
            nc.scalar.activation(
                e_sb[:, 0:PEB],
                psum_l[:, :],
                mybir.ActivationFunctionType.Exp,
                scale=1.0 / S2,
                accum_out=esum2[:, 0:1],
            )
            nc.scalar.activation(
                e_sb[:, PEB:NB],
                lg[:, :],
                mybir.ActivationFunctionType.Exp,
                scale=1.0 / S2,
                accum_out=esum2[:, 1:2],
            )
            psum_s = ppool.tile([1, 2], F32, tag="ps")
            nc.tensor.matmul(psum_s[:, :], ones128[:, :], esum2[:, :])
            ls = cpool.tile([1, 1], F32, tag="ls")
            nc.vector.tensor_reduce(
                ls[:, :], psum_s[:, :], mybir.AxisListType.X, mybir.AluOpType.add
            )

            cc2_in = dpool.tile([1, 1], F32, tag="cc2_in")
            cc2_out = dpool.tile([1, W], F32, tag="cc2_out")
            nc.gpsimd.dma_start(cc2_in[:, :], ls[:, :])
            nc.gpsimd.collective_compute(
                "AllGather",
                mybir.AluOpType.bypass,
                replica_groups=[list(range(W))],
                ins=[cc2_in.opt()],
                outs=[cc2_out.opt()],
            )
            # broadcast-read the gathered sums to all partitions in one DMA
            sall = cpool.tile([128, W], F32, tag="sall")
            nc.gpsimd.dma_start(
                sall[:, :], cc2_out[:, :].broadcast_to([128, W])
            )
            tsum = cpool.tile([128, 1], F32, tag="tsum")
            nc.vector.tensor_reduce(
                tsum[:, :], sall[:, :], mybir.AxisListType.X, mybir.AluOpType.add
            )
            rb = cpool.tile([128, 1], F32, tag="rb")
            nc.vector.reciprocal(rb[:, :], tsum[:, :])

            # final scale on ScalarE (vector may still be draining; scalar
            # is idle after the exps)
            y_sb = cpool.tile([128, NB], F32, tag="ysb")
            nc.scalar.activation(
                y_sb[:, :],
                e_sb[:, :],
                mybir.ActivationFunctionType.Copy,
                scale=rb[:, 0:1],
            )
            nc.gpsimd.dma_start(y_out[:, :], y_sb[:, :])

    _split_multi_waits(nc)
    return nc


_NC_CACHE = None


def _get_nc():
    global _NC_CACHE
    if _NC_CACHE is None:
        _NC_CACHE = build_kernel()
    return _NC_CACHE


def _prep_inputs(context_words, W_in, W_out):
    """Host-side shard + layout prep (pure data movement + dtype cast)."""
    in_maps = []
    cw = np.asarray(context_words, dtype=np.float32)
    wi = np.asarray(W_in, dtype=np.float32)
    wo = np.asarray(W_out, dtype=np.float32)
    for r in range(W):
        v0 = r * VL
        ctx_s = cw[:, v0:v0 + VL].astype(NP_BF16)
        # ctxp[p, j*C + c] = ctx[c, 128j + p]
        ctxp = np.ascontiguousarray(
            ctx_s.reshape(C, NJ, 128).transpose(2, 1, 0).reshape(128, NJ * C)
        )
        # w1t[p, j*N + n] = W_in[n, v0 + 128j + p]*S1  (partition-major pack)
        w1t = np.ascontiguousarray(
            (wi[:, v0:v0 + VL].T * np.float32(S1)).astype(NP_FP8)
            .reshape(NJ, 128, N).transpose(1, 0, 2).reshape(128, NJ * N)
        )
        # ws[p, b, n] = W_out[v0 + 125p + b, n]*S2
        ws = (wo[v0:v0 + VL, :] * np.float32(S2)).astype(NP_FP8).reshape(128, NB, N)
        # PE half: w2p[n, 128b + p] = ws[p, b, n], b < PEB
        w2p = np.ascontiguousarray(
            ws[:, :PEB, :].transpose(2, 1, 0).reshape(N, PEB * 128)
        )
        # DVE+GP lanes: w2r[p, bb*N + n] = ws[p, PEB+bb, n]
        w2r = np.ascontiguousarray(ws[:, PEB:, :].reshape(128, (DVB + GPB) * N))
        in_maps.append({"ctxp": ctxp, "w1t": w1t, "w2p": w2p, "w2r": w2r})
    return in_maps


def kernel(context_words, W_in, W_out):
    nc = _get_nc()
    in_maps = _prep_inputs(context_words, W_in, W_out)
    res = run_bass_kernel_spmd(nc, in_maps, list(range(W)))
    # y[p, b] on core r = prob[r*VL + 125*p + b]
    return np.concatenate(
        [np.asarray(res.results[r]["y"], dtype=np.float32).reshape(VL) for r in range(W)]
    )


# revision 18
# speedup vs baseline: 1.5177x; 1.5177x over previous
"""CBOW (nn_CBOW_88991722373900) Trainium2 kernel, v2.

Full-input contract: kernel(context_words[10,128000] f32, W_in[300,128000] f32,
W_out[128000,300] f32) -> softmax probabilities [128000] f32.

Strategy (8-way tensor/model parallel over the vocab dim V):
  - shard V into 8 chunks of 16000; each core holds its slice of both weight
    matrices cast to fp8e4 on host (memory-bound problem: quarter of the f32
    HBM traffic). Weights are pre-scaled by powers of two (S1, S2) so the
    fp8 values sit in the normal range; descales fold into existing scalar
    ops on the f32 PSUM results.
  - ctx pre-reduced over the C=10 window on DVE -> s[128,125] bf16, so
    GEMM1 is s-chunk (stationary, [128,1] bf16) x W_inT tile (moving fp8,
    [128,300]) accumulated over 125 v-chunks into PSUM [1,300].
  - trigger path: PSUM->SBUF bf16 copy -> gpsimd DMA -> AllGather(600B)
    (gpsimd carries only collective-path + output traffic, so the trigger
    fires as soon as GEMM1 retires).
  - post-AG rank-sum directly on PE in both layouts GEMM2 needs:
      h_nt  [128,3]  = hall-chunk (stationary) x ones8 (moving)   (n on parts)
      h_rep [128,300] = ones[8,128] (stationary) x hall (moving)  (broadcast)
    exact f32 1/(C*S1) on the PSUM->SBUF copies.
  - GEMM2 split across PE and DVE:
      PE  (v-blocks 0..PEB):  w2p fp8 col-blocks stationary (FWL), h_nt bf16
          moving -> PSUM [128, PEB]
      DVE (v-blocks PEB..125): one fused tensor_tensor_reduce per block:
          (w2r fp8 * h_rep bf16) summed over n -> lg_dve [128, DVB] f32
    v mapped so partition p holds contiguous v = 125*p + b
  - softmax: exp on ScalarE with scale=1/S2 (no max subtraction: |logit| < ~1
    at these weight scales), local sum via ones-matmul, AllGather(4B) for the
    global denominator, scale, DMA out.
"""

import numpy as np
import ml_dtypes

import concourse.bass as bass
import concourse.mybir as mybir
from concourse import tile
from concourse.bass_utils import run_bass_kernel_spmd
from concourse.vector_clock import ScopedClock, VectorClock

V = 128000
N = 300
C = 10
W = 8              # cores
VL = V // W        # 16000 vocab per core
NJ = VL // 128     # 125 v-chunks for GEMM1
NB = VL // 128     # 125 v-blocks for GEMM2
NCH = [(0, 128), (128, 128), (256, 44)]  # n-chunks
PEB = 72           # v-blocks on the PE lane of GEMM2
DVB = 33           # v-blocks on the DVE lane
GPB = NB - PEB - DVB  # v-blocks on the GpSimd lane
S1 = 4096.0        # host pre-scale on W_in  (values ~2.8e-3 -> ~11.4)
S2 = 128.0         # host pre-scale on W_out (values ~0.058  -> ~7.4)

BF16 = mybir.dt.bfloat16
F32 = mybir.dt.float32
FP8 = mybir.dt.float8e4
NP_BF16 = ml_dtypes.bfloat16
NP_FP8 = ml_dtypes.float8_e4m3fn

# w1 DMA groups: first ones small for an early GEMM1 start
W1_GROUPS = [4, 6] + [10] * 11 + [5]  # sums to 125
# DVE lane: one grouped-3D chunk (A) + per-block 2D chunks (B) to compare
# styles in the trace; GP lane: grouped-3D on gpsimd.
W2R_GROUPS = [33, 10, 10]             # DMA groups; sums to DVB + GPB


def _patched_drain_and_barrier(self, tick_clock, wait_clock):
    """Tail-drain waits split into 1-wait NOPs: this walrus build's CTRL
    instructions only encode a single sync wait."""
    vc = tick_clock.global_clock
    procs = [(p, vc[p]) for p in range(len(vc)) if vc[p] > 0]
    for i, (p, t) in enumerate(procs):
        pvc = VectorClock([0] * len(vc))
        pvc.require_at_least(p, t)
        nop_inst = self.nc.sync.nop(nofuse=True, hint=f"tail_wait_{i}")
        wait_clock.add_sem_waits(nop_inst.ins, ScopedClock({None: pvc}))
    self.nc.sync.drain()
    self.nc.all_engine_barrier(sem_only=True)
    assert self.sems is not None
    popped = self.nc._tile_sem_poison_stack.pop()
    assert popped is self._sem_poison
    self.nc.clear_and_free_semaphores(list(self.sems.allocated().values()))
    self.nc.all_engine_barrier(sem_only=True)


tile.TileContext._drain_and_barrier = _patched_drain_and_barrier


def _split_multi_waits(nc):
    """This walrus build encodes at most ONE sync wait per instruction. Hoist
    excess waits onto same-engine NoOps inserted immediately before."""
    import bass_rust

    ctr = [0]

    def make_nop(engine, wait):
        ctr[0] += 1
        nop = mybir.InstNoOp(name=f"I-wsplit{ctr[0]}", engine=engine)
        nop.bass_nofuse = True
        nop.sync_info = bass_rust.SyncInfo(on_wait=[wait], on_update=[])
        nc.register_instruction(nop, overwrite=True)
        return nop

    for bb in nc.main_func.blocks:
        out = []
        for ins in bb.instructions:
            si = ins.sync_info
            if si is not None and si.on_wait and len(si.on_wait) > 1:
                waits = list(si.on_wait)
                for w in waits[:-1]:
                    out.append(make_nop(ins.engine, w))
                ins.sync_info = bass_rust.SyncInfo(
                    on_wait=[waits[-1]], on_update=list(si.on_update)
                )
            out.append(ins)
        bb.instructions = out


def build_kernel():
    nc = bass.Bass()

    ctxp = nc.dram_tensor("ctxp", [128, NJ * C], BF16, kind="ExternalInput")
    # w1t packed partition-major on host: w1t[p, j*N + n] = W_in[n, v0+128j+p]*S1
    w1t = nc.dram_tensor("w1t", [128, NJ * N], FP8, kind="ExternalInput")
    # w2p: PE half, w2p[n, 128b+p] = W_out[v0+125p+b, n]*S2, b in [0, PEB)
    w2p = nc.dram_tensor("w2p", [N, PEB * 128], FP8, kind="ExternalInput")
    # w2r: DVE+GP lanes, w2r[p, bb*N+n] = W_out[v0+125p+PEB+bb, n]*S2
    w2r = nc.dram_tensor("w2r", [128, (DVB + GPB) * N], FP8, kind="ExternalInput")
    y_out = nc.dram_tensor("y", [128, NB], F32, kind="ExternalOutput")

    with tile.TileContext(nc) as tc:
        with (
            tc.tile_pool(name="const", bufs=1) as cpool,
            tc.tile_pool(name="scr", bufs=2) as spool,
            tc.tile_pool(name="psum", bufs=1, space="PSUM") as ppool,
            tc.tile_pool(name="dram", bufs=1, space="DRAM") as dpool,
        ):
            # ---- constants ----
            ones8 = cpool.tile([W, 1], F32, tag="ones8")
            nc.vector.memset(ones8[:, :], 1.0)
            ones8r = cpool.tile([W, 128], F32, tag="ones8r")
            nc.vector.memset(ones8r[:, :], 1.0)
            ones128 = cpool.tile([128, 1], F32, tag="ones128")
            nc.vector.memset(ones128[:, :], 1.0)

            # ---- warmup collective: a 4B AllGather with no data deps,
            # triggered at ~10us. It absorbs the cross-core launch stagger
            # and the ncfw cold-start while GEMM1/DMA run, so the real
            # collectives hit a warm, rank-synchronized cc queue. ----
            ccw_in = dpool.tile([1, 1], F32, tag="ccw_in")
            ccw_out = dpool.tile([W, 1], F32, tag="ccw_out")
            nc.gpsimd.dma_start(ccw_in[:, :], ones128[0:1, 0:1])
            nc.gpsimd.collective_compute(
                "AllGather",
                mybir.AluOpType.bypass,
                replica_groups=[list(range(W))],
                ins=[ccw_in.opt()],
                outs=[ccw_out.opt()],
            )

            # ---- input DMA streams ----
            # W_in (and ctx) strictly ahead of W_out on both rings so every
            # core's GEMM1 inputs win HBM arbitration fleet-wide; W_out
            # streams during the AllGather window.
            ctx_sb = cpool.tile([128, NJ * C], BF16, tag="ctx")
            JH = 63  # ctx/s split point
            nc.sync.dma_start(ctx_sb[:, 0:JH * C], ctxp[:, 0:JH * C])
            nc.scalar.dma_start(ctx_sb[:, JH * C:], ctxp[:, JH * C:])

            w1_sb = []
            j0 = 0
            for g, nj in enumerate(W1_GROUPS):
                t = cpool.tile([128, nj * N], FP8, tag=f"w1_{g}")
                ring = nc.scalar if g % 2 == 0 else nc.sync
                ring.dma_start(t[:, :], w1t[:, j0 * N:(j0 + nj) * N])
                w1_sb.append((t, j0, nj))
                j0 += nj

            w2p_sb = []
            for i3, (off, kk) in enumerate(NCH):
                t = cpool.tile([kk, PEB * 128], FP8, tag=f"w2p_{i3}")
                ring = nc.sync if i3 == 1 else nc.scalar
                ring.dma_start(t[:, :], w2p[off:off + kk, :])
                w2p_sb.append(t)

            w2r_sb = []
            bb0 = 0
            for g, nb in enumerate(W2R_GROUPS):
                t = cpool.tile([128, nb * N], FP8, tag=f"w2r_{g}")
                ring = nc.sync if g % 2 == 0 else nc.scalar
                ring.dma_start(t[:, :], w2r[:, bb0 * N:(bb0 + nb) * N])
                w2r_sb.append((t, bb0, nb))
                bb0 += nb

            # ---- ctx pre-reduce over C -> s[128, 125] bf16 ----
            s_sb = cpool.tile([128, NJ], BF16, tag="s")
            with nc.allow_low_precision(reason="C=10 window sum in bf16"):
                nc.vector.tensor_reduce(
                    s_sb[:, 0:JH],
                    ctx_sb[:, 0:JH * C].rearrange("p (j c) -> p j c", j=JH),
                    mybir.AxisListType.X,
                    mybir.AluOpType.add,
                )
                nc.vector.tensor_reduce(
                    s_sb[:, JH:],
                    ctx_sb[:, JH * C:].rearrange("p (j c) -> p j c", j=NJ - JH),
                    mybir.AxisListType.X,
                    mybir.AluOpType.add,
                )

            # ---- GEMM1: psum_hl[0, n] += s_j^T x w1 tile ----
            psum_hl = ppool.tile([1, N], F32, tag="phl")
            for t, j0g, nj in w1_sb:
                for jj in range(nj):
                    j = j0g + jj
                    nc.tensor.matmul(
                        psum_hl[:, :],
                        s_sb[:, j:j + 1],
                        t[:, jj * N:(jj + 1) * N],
                        start=(j == 0),
                        stop=(j == NJ - 1),
                    )

            # local partial hidden (scaled by S1*C) -> AllGather 1.2KB
            h_loc = cpool.tile([1, N], F32, tag="hloc")
            nc.vector.tensor_copy(h_loc[:, :], psum_hl[:, :])
            cc_in = dpool.tile([1, N], F32, tag="cc_in")
            cc_out = dpool.tile([W, N], F32, tag="cc_out")
            nc.gpsimd.dma_start(cc_in[:, :], h_loc[:, :])
            nc.gpsimd.collective_compute(
                "AllGather",
                mybir.AluOpType.bypass,
                replica_groups=[list(range(W))],
                ins=[cc_in.opt()],
                outs=[cc_out.opt()],
            )
            hall = cpool.tile([W, N], F32, tag="hall")
            nc.gpsimd.dma_start(hall[:, :], cc_out[:, :])

            # ---- rank-sum on PE, directly in both layouts GEMM2 needs ----
            # h_nt: n on partitions ([128, 3] col per n-chunk)
            psum_t = ppool.tile([128, 3], F32, tag="pt")
            for i3, (off, kk) in enumerate(NCH):
                nc.tensor.matmul(
                    psum_t[0:kk, i3:i3 + 1],
                    hall[:, off:off + kk],
                    ones8[:, :],
                    start=True,
                    stop=True,
                )
            # h_rep: partition-broadcast of the rank sum
            psum_r = ppool.tile([128, N], F32, tag="pr")
            nc.tensor.matmul(psum_r[:, :], ones8r[:, :], hall[:, :])

            h_nt = cpool.tile([128, 3], BF16, tag="hnt")
            nc.vector.tensor_scalar_mul(h_nt[:, :], psum_t[:, :], 1.0 / (C * S1))
            h_rep = cpool.tile([128, N], BF16, tag="hrep")
            nc.scalar.activation(
                h_rep[:, :],
                psum_r[:, :],
                mybir.ActivationFunctionType.Copy,
                scale=1.0 / (C * S1),
            )

            # ---- GEMM2 PE half: logits[p, b]*S2 for b in [0, PEB) ----
            psum_l = ppool.tile([128, PEB], F32, tag="pl")
            for b in range(PEB):
                for i3, (off, kk) in enumerate(NCH):
                    nc.tensor.matmul(
                        psum_l[:, b:b + 1],
                        w2p_sb[i3][:, b * 128:(b + 1) * 128],
                        h_nt[0:kk, i3:i3 + 1],
                        start=(i3 == 0),
                        stop=(i3 == 2),
                    )

            # ---- GEMM2 DVE + GpSimd lanes over lg[128, DVB+GPB] ----
            # DVE lane: first DVE_G3D blocks grouped-3D, rest per-block 2D
            # (style A/B comparison in the trace); GP lane: grouped-3D.
            lg = cpool.tile([128, DVB + GPB], F32, tag="lg")
            h_b = h_rep[:, :].rearrange("p (x n) -> p x n", x=1)
            for t, bb0, nb in w2r_sb:
                if bb0 >= DVB:
                    # GP lane: grouped mul on gpsimd, per-block reduce on
                    # ScalarE via activation accum_out
                    scr = spool.tile([128, nb * N], BF16, tag="gp_scr")
                    nc.gpsimd.tensor_mul(
                        scr[:, :].rearrange("p (b n) -> p b n", b=nb),
                        t[:, 0:nb * N].rearrange("p (b n) -> p b n", b=nb),
                        h_b.broadcast_to([128, nb, N]),
                    )
                    for bb in range(nb):
                        dump = spool.tile([128, N], BF16, tag="gp_dump")
                        nc.scalar.activation(
                            dump[:, :],
                            scr[:, bb * N:(bb + 1) * N],
                            mybir.ActivationFunctionType.Copy,
                            accum_out=lg[:, bb0 + bb:bb0 + bb + 1],
                        )
                else:
                    # DVE lane: one big grouped mul + segmented reduce
                    scr = spool.tile([128, nb * N], BF16, tag="dve_scr")
                    nc.vector.tensor_mul(
                        scr[:, :].rearrange("p (b n) -> p b n", b=nb),
                        t[:, 0:nb * N].rearrange("p (b n) -> p b n", b=nb),
                        h_b.broadcast_to([128, nb, N]),
                    )
                    nc.vector.tensor_reduce(
                        lg[:, bb0:bb0 + nb],
                        scr[:, :].rearrange("p (b n) -> p b n", b=nb),
                        mybir.AxisListType.X,
                        mybir.AluOpType.add,
                    )

            # ---- softmax ----
            e_sb = cpool.tile([128, NB], F32, tag="esb")
            esum2 = cpool.tile([128, 2], F32, tag="esum2")
            nc.scalar.activation(
                e_sb[:, 0:PEB],
                psum_l[:, :],
                mybir.ActivationFunctionType.Exp,
                scale=1.0 / S2,
                accum_out=esum2[:, 0:1],
            )
            nc.scalar.activation(
                e_sb[:, PEB:NB],
                lg[:, :],
                mybir.ActivationFunctionType.Exp,
                scale=1.0 / S2,
                accum_out=esum2[:, 1:2],
            )
            psum_s = ppool.tile([1, 2], F32, tag="ps")
            nc.tensor.matmul(psum_s[:, :], ones128[:, :], esum2[:, :])

            ls2 = cpool.tile([1, 2], F32, tag="ls2")
            nc.vector.tensor_copy(ls2[:, :], psum_s[:, :])
            cc2_in = dpool.tile([1, 2], F32, tag="cc2_in")
            cc2_out = dpool.tile([1, 2 * W], F32, tag="cc2_out")
            nc.gpsimd.dma_start(cc2_in[:, :], ls2[:, :])
            nc.gpsimd.collective_compute(
                "AllGather",
                mybir.AluOpType.bypass,
                replica_groups=[list(range(W))],
                ins=[cc2_in.opt()],
                outs=[cc2_out.opt()],
            )
            # broadcast-read the gathered sums to all partitions in one DMA
            sall = cpool.tile([128, 2 * W], F32, tag="sall")
            nc.gpsimd.dma_start(
                sall[:, :], cc2_out[:, :].broadcast_to([128, 2 * W])
            )
            tsum = cpool.tile([128, 1], F32, tag="tsum")
            nc.vector.tensor_reduce(
                tsum[:, :], sall[:, :], mybir.AxisListType.X, mybir.AluOpType.add
            )
            rb = cpool.tile([128, 1], F32, tag="rb")
            nc.vector.reciprocal(rb[:, :], tsum[:, :])

            # final scale on ScalarE (vector may still be draining; scalar
            # is idle after the exps)
            y_sb = cpool.tile([128, NB], F32, tag="ysb")
            nc.scalar.activation(
                y_sb[:, :],
                e_sb[:, :],
                mybir.ActivationFunctionType.Copy,
                scale=rb[:, 0:1],
            )
            nc.gpsimd.dma_start(y_out[:, :], y_sb[:, :])

    _split_multi_waits(nc)
    return nc


_NC_CACHE = None


def _get_nc():
    global _NC_CACHE
    if _NC_CACHE is None:
        _NC_CACHE = build_kernel()
    return _NC_CACHE


def _prep_inputs(context_words, W_in, W_out):
    """Host-side shard + layout prep (pure data movement + dtype cast)."""
    in_maps = []
    cw = np.asarray(context_words, dtype=np.float32)
    wi = np.asarray(W_in, dtype=np.float32)
    wo = np.asarray(W_out, dtype=np.float32)
    for r in range(W):
        v0 = r * VL
        ctx_s = cw[:, v0:v0 + VL].astype(NP_BF16)
        # ctxp[p, j*C + c] = ctx[c, 128j + p]
        ctxp = np.ascontiguousarray(
            ctx_s.reshape(C, NJ, 128).transpose(2, 1, 0).reshape(128, NJ * C)
        )
        # w1t[p, j*N + n] = W_in[n, v0 + 128j + p]*S1  (partition-major pack)
        w1t = np.ascontiguousarray(
            (wi[:, v0:v0 + VL].T * np.float32(S1)).astype(NP_FP8)
            .reshape(NJ, 128, N).transpose(1, 0, 2).reshape(128, NJ * N)
        )
        # ws[p, b, n] = W_out[v0 + 125p + b, n]*S2
        ws = (wo[v0:v0 + VL, :] * np.float32(S2)).astype(NP_FP8).reshape(128, NB, N)
        # PE half: w2p[n, 128b + p] = ws[p, b, n], b < PEB
        w2p = np.ascontiguousarray(
            ws[:, :PEB, :].transpose(2, 1, 0).reshape(N, PEB * 128)
        )
        # DVE+GP lanes: w2r[p, bb*N + n] = ws[p, PEB+bb, n]
        w2r = np.ascontiguousarray(ws[:, PEB:, :].reshape(128, (DVB + GPB) * N))
        in_maps.append({"ctxp": ctxp, "w1t": w1t, "w2p": w2p, "w2r": w2r})
    return in_maps


def kernel(context_words, W_in, W_out):
    nc = _get_nc()
    in_maps = _prep_inputs(context_words, W_in, W_out)
    res = run_bass_kernel_spmd(nc, in_maps, list(range(W)))
    # y[p, b] on core r = prob[r*VL + 125*p + b]
    return np.concatenate(
        [np.asarray(res.results[r]["y"], dtype=np.float32).reshape(VL) for r in range(W)]
    )
